# revision 31
# baseline (speedup 1.0000x reference)
"""Mistral attention (B=1, S=2048, H=4096, 32 q-heads / 8 kv-heads GQA,
RoPE, causal) on 8 trn2 NeuronCores.

Sharding: tensor-parallel by kv head. Core c owns kv head c, q heads
4c..4c+3, and Wo rows 512c..512c+512. Attention outputs are AllGathered
per 512-token chunk; each core then computes its 512-row slice of the
output projection.

Schedule: chunk-interleaved. Token chunks are processed in pairs
(0,1) then (2,3); for each pair the QKV projections run as two passes
(KV then Q) with each weight tile kept stationary on the PE for both
chunks' matmuls (halves LDWEIGHTS traffic). Attention for chunk c runs
right after its projections, and its AllGather fires immediately — the
four AllGathers cascade behind the remaining projection/attention
compute instead of bunching at the end. The output projection runs last
as two 2-chunk passes (weight tile stationary across both chunks).

Attention inner loop is kt-outer/head-inner so each K/V tile is loaded
once per chunk for 4 q-heads. Scores/exp/AV on masked diagonal tiles
are trimmed to live query columns (N = 512-128m). Softmax denominators
accumulate on the vector engine (elementwise over kt tiles) with a
single K=128 matmul per head at the end; the reciprocal is broadcast
across partitions with a K=1 f32r matmul. Softmax skips
max-subtraction (unit-scale inputs). Value path runs bf16; PSUM
accumulation fp32.
"""

import math

import ml_dtypes
import numpy as np

P = 128
S = 2048
H = 4096
HD = 128
NQH = 4  # q heads per core
TC = 512  # token chunk
NT = S // TC  # 4 chunks
HT = H // P  # 32 h tiles
N_CORES = 8
ROPE_THETA = 10000.0

_BUILT = None


def _rope_tables():
    """cosT/sin2T in [hd partition, token free] layout.

    sin2T is the sin table pre-shifted/signed so that
    q_rot = q*cosT + shift128(q*sin2T), where shift128 swaps the two
    64-partition halves.
    """
    inv_freq = 1.0 / (ROPE_THETA ** (np.arange(0, HD, 2, dtype=np.float64) / HD))
    t = np.arange(S, dtype=np.float64)
    freqs = np.outer(t, inv_freq)  # [S, 64]
    emb = np.concatenate([freqs, freqs], axis=1)  # [S, HD]
    cosT = np.cos(emb).T.astype(np.float32)  # [HD, S]
    sinT = np.sin(emb).T.astype(np.float32)
    sin2T = sinT.copy()
    sin2T[64:] = -sin2T[64:]
    return (
        np.ascontiguousarray(cosT).astype(ml_dtypes.bfloat16),
        np.ascontiguousarray(sin2T).astype(ml_dtypes.bfloat16),
    )


def _mask():
    """[128, 512] bf16: mask[i, j] = (j >= i). Diagonal tile m of a chunk
    uses mask[:, 0:512-128m] against query columns [128m, 512)."""
    i = np.arange(P)[:, None]
    j = np.arange(TC)[None, :]
    return np.ascontiguousarray((j >= i).astype(np.float32)).astype(ml_dtypes.bfloat16)


def _build():
    import concourse.bacc as bacc
    import concourse.mybir as mybir
    import concourse.tile as tile

    f32 = mybir.dt.float32
    f32r = mybir.dt.float32r
    bf16 = mybir.dt.bfloat16

    nc = bacc.Bacc(
        "TRN2", target_bir_lowering=False, debug=False, num_devices=N_CORES
    )

    # Host-side repacked layouts: partition-major [128, ...] with wide
    # contiguous rows so DMA descriptors are 2KB+ (1KB rows cap a DMA
    # queue at ~90GB/s; the kernel front is load-bound otherwise).
    hs2 = nc.declare_dram_parameter("hs2", [P, HT, S], bf16, isOutput=False)
    wq2 = nc.declare_dram_parameter("wq2", [P, HT * NQH * HD], bf16, isOutput=False)
    wk2 = nc.declare_dram_parameter("wk2", [P, HT * HD], bf16, isOutput=False)
    wv2 = nc.declare_dram_parameter("wv2", [P, HT * HD], bf16, isOutput=False)
    wo2 = nc.declare_dram_parameter("wo2", [P, HT * NQH * HD], bf16, isOutput=False)
    out_ext = nc.declare_dram_parameter("out", [NQH * HD, S], f32, isOutput=True)

    cosT_np, sin2T_np = _rope_tables()
    cos_dram = nc.inline_tensor(cosT_np, name="cosT")
    sin_dram = nc.inline_tensor(sin2T_np, name="sin2T")
    mask_dram = nc.inline_tensor(_mask(), name="mask")
    id_dram = nc.inline_tensor(np.eye(P).astype(ml_dtypes.bfloat16), name="ident")

    # rank-major AllGather payload: [hd, (head, token)] per rank, so the
    # o-proj phase can read one rank's 4 heads as a single 4KB-row DMA
    ag_in = [nc.dram_tensor(f"ag_in{c}", [P, NQH * TC], bf16) for c in range(NT)]
    ag_out = [
        nc.dram_tensor(f"ag_out{c}", [N_CORES * P, NQH * TC], bf16, addr_space="Shared")
        for c in range(NT)
    ]

    Exp = mybir.ActivationFunctionType.Exp
    SCALE = 1.0 / math.sqrt(HD)

    with tile.TileContext(nc) as tc:
        with (
            tc.tile_pool(name="const", bufs=1) as constp,
            tc.tile_pool(name="qkvout", bufs=1) as qp,
            tc.tile_pool(name="pmain", bufs=1, space="PSUM") as pm,
        ):
            # constants
            cos_sb = constp.tile([P, S], bf16)
            sin_sb = constp.tile([P, S], bf16)
            mask_sb = constp.tile([P, TC], bf16)
            mask2_sb = constp.tile([P, 2 * TC], bf16)
            ones_sb = constp.tile([P, 1], bf16)
            onesrow_sb = constp.tile([1, P], bf16)
            id_sb = constp.tile([P, P], bf16)
            nc.gpsimd.memset(ones_sb[:], 1.0)
            nc.gpsimd.memset(onesrow_sb[:], 1.0)

            # persistent qkv outputs (bf16: PE runs bf16 at full rate)
            qT_sb = qp.tile([P, NQH * S], bf16)  # [hd, (head, t)]
            kT_sb = qp.tile([P, S], bf16)
            vnat_sb = qp.tile([P, S], bf16)  # [t%128, (ttile, hd)]

            # PSUM: 8 banks as two 2-bank tiles (p01, p23) and four 1-bank
            # tiles (pa..pd). Explicit tags keep cross-phase deps per-bank.
            def p2(tag, name):
                return pm.tile([P, 2 * TC], f32, tag=tag, bufs=1, name=name)

            def p1(tag, name):
                return pm.tile([P, TC], f32, tag=tag, bufs=1, name=name)

            with (
                tc.tile_pool(name="wqkv", bufs=1) as wp,
                tc.tile_pool(name="hsp", bufs=4) as hsp,
                tc.tile_pool(name="workA", bufs=2) as workp,
            ):
                wq_sb = wp.tile([P, HT * NQH * HD], bf16)
                wk_sb = wp.tile([P, HT * HD], bf16)
                wv_sb = wp.tile([P, HT * HD], bf16)

                def attn(c):
                    """Attention for chunk c + its AllGather."""
                    nkt = 4 * (c + 1)
                    avt = ["pa", "pb", "pc", "pd"]
                    av = [p1(avt[h], f"av_{c}_{h}") for h in range(NQH)]
                    ds = [
                        workp.tile([P, TC], bf16, tag=f"ds{h}", bufs=1,
                                   name=f"ds_{c}_{h}")
                        for h in range(NQH)
                    ]
                    pend = None  # (exs, coff, ncols, kt) awaiting AV matmuls

                    def emit_av(p):
                        exv, coff, kt = p
                        for h in range(NQH):
                            nc.tensor.matmul(
                                av[h][:, coff:TC],
                                vnat_sb[:, kt * P : (kt + 1) * P],
                                exv[h],
                                start=(kt == 0),
                                stop=(kt == nkt - 1),
                            )

                    for kt in range(nkt):
                        m = kt - 4 * c
                        ncols = TC - 128 * m if m > 0 else TC
                        coff = TC - ncols
                        scp = p2("p01", f"scp_{c}_{kt}")
                        scq = p2("p23", f"scq_{c}_{kt}")
                        halves = [
                            scp[:, 0:TC], scp[:, TC : 2 * TC],
                            scq[:, 0:TC], scq[:, TC : 2 * TC],
                        ]
                        for h in range(NQH):
                            nc.tensor.matmul(
                                halves[h][:, coff:TC],
                                kT_sb[:, kt * P : (kt + 1) * P],
                                qT_sb[:, h * S + c * TC + coff : h * S + (c + 1) * TC],
                                start=True,
                                stop=True,
                            )
                        if pend is not None:
                            emit_av(pend)
                        ex01 = workp.tile([P, 2 * TC], bf16, tag="ex", bufs=4,
                                          name=f"ex01_{c}_{kt}")
                        ex23 = workp.tile([P, 2 * TC], bf16, tag="ex", bufs=4,
                                          name=f"ex23_{c}_{kt}")
                        if coff == 0:
                            nc.scalar.activation(ex01[:], scp[:], Exp, scale=SCALE)
                            nc.scalar.activation(ex23[:], scq[:], Exp, scale=SCALE)
                        else:
                            for ex, sc in ((ex01, scp), (ex23, scq)):
                                nc.scalar.activation(
                                    ex[:, 0:ncols], sc[:, coff:TC], Exp, scale=SCALE
                                )
                                nc.scalar.activation(
                                    ex[:, TC : TC + ncols], sc[:, TC + coff : 2 * TC],
                                    Exp, scale=SCALE,
                                )
                        exv = [ex01[:, 0:ncols], ex01[:, TC : TC + ncols],
                               ex23[:, 0:ncols], ex23[:, TC : TC + ncols]]
                        if m == 0:
                            nc.vector.tensor_mul(ex01[:], ex01[:], mask2_sb[:])
                            nc.vector.tensor_mul(ex23[:], ex23[:], mask2_sb[:])
                        elif m > 0:
                            for h in range(NQH):
                                nc.vector.tensor_mul(
                                    exv[h], exv[h], mask_sb[:, 0:ncols]
                                )
                        for h in range(NQH):
                            if kt == 0:
                                nc.vector.tensor_copy(ds[h][:], exv[h])
                            else:
                                nc.vector.tensor_add(
                                    ds[h][:, coff:TC], ds[h][:, coff:TC], exv[h]
                                )
                        pend = (exv, coff, kt)
                    emit_av(pend)

                    # per-head tail: dn -> 1/dn -> broadcast -> normalize.
                    # dn/bc live in p01/p23 halves (the next phase needs
                    # those banks only at its Q pass, ~27us later). Heads
                    # 0-1 emit here; heads 2-3 + the AllGather return as a
                    # closure the caller weaves into the next phase's
                    # instruction stream (avoids PE-FIFO head-of-line).
                    avss = []
                    for h in range(NQH):
                        avs = workp.tile([P, TC], f32, tag=f"avs{h}", bufs=1,
                                         name=f"avs_{c}_{h}")
                        nc.scalar.copy(avs[:], av[h][:])
                        avss.append(avs)

                    def tail_pair(pair):
                        dnt = {}
                        for h in pair:
                            t = p1(("pc", "pd")[h % 2], f"dn_{c}_{h}")
                            nc.tensor.matmul(
                                t[0:1, :], ones_sb[:], ds[h][:],
                                start=True, stop=True,
                            )
                            dnt[h] = t
                        rcbs = {}
                        for h in pair:
                            rc = workp.tile([1, TC], f32, tag="rc", bufs=4,
                                            name=f"rc_{c}_{h}")
                            nc.vector.reciprocal_approx_fast(rc[:], dnt[h][0:1, :])
                            rcb = workp.tile([1, TC], bf16, tag="rcb", bufs=4,
                                             name=f"rcb_{c}_{h}")
                            nc.vector.tensor_copy(rcb[:], rc[:])
                            rcbs[h] = rcb
                        bct = {}
                        for h in pair:
                            t = p1(("pc", "pd")[h % 2], f"bc_{c}_{h}")
                            nc.tensor.matmul(
                                t[:], onesrow_sb[:], rcbs[h][:],
                                start=True, stop=True,
                            )
                            bct[h] = t
                        for h in pair:
                            ao = workp.tile([P, TC], bf16, tag="ao", bufs=4,
                                            name=f"ao_{c}_{h}")
                            nc.vector.tensor_mul(
                                ao[:], avss[h][:], bct[h][:]
                            )
                            nc.sync.dma_start(
                                out=ag_in[c][:, h * TC : (h + 1) * TC], in_=ao[:]
                            )

                    tail_pair((0, 1))

                    def finish():
                        tail_pair((2, 3))
                        nc.gpsimd.collective_compute(
                            "AllGather",
                            mybir.AluOpType.bypass,
                            ins=[ag_in[c][:]],
                            outs=[ag_out[c][:]],
                            replica_groups=[list(range(N_CORES))],
                        )

                    return finish

                def emit_loads(pi, ca, cb):
                    """hs for the pair: four [128, 8x1024] supertile DMAs
                    (2KB rows, each dma_start split across 16 SDMA engines).
                    Weights (first pair only): single big contiguous DMAs."""
                    hs_t = {}
                    for g in range(0, HT, 8):
                        t = hsp.tile([P, 8 * 2 * TC], bf16, tag="hs",
                                     name=f"hs_{ca}_{g}")
                        eng = nc.scalar if pi else (nc.sync if g % 16 == 0 else nc.scalar)
                        if pi == 0 and g == 0:
                            # startup-critical order: the Q pass runs first,
                            # so the first wq half leads the sync queue while
                            # hs streams on scalar; wk/wv follow
                            half = HT * NQH * HD // 2
                            nc.sync.dma_start(
                                out=wq_sb[:, 0:half], in_=wq2[:, 0:half]
                            )
                            nc.scalar.dma_start(
                                out=t[:, 0 : 4 * 1024],
                                in_=hs2[:, 0:4, ca * TC : (cb + 1) * TC],
                            )
                            nc.scalar.dma_start(
                                out=t[:, 4 * 1024 : 8 * 1024],
                                in_=hs2[:, 4:8, ca * TC : (cb + 1) * TC],
                            )
                            nc.sync.dma_start(
                                out=wq_sb[:, half : 2 * half],
                                in_=wq2[:, half : 2 * half],
                            )
                            nc.scalar.dma_start(out=wk_sb[:], in_=wk2[:])
                            nc.sync.dma_start(out=wv_sb[:], in_=wv2[:])
                        else:
                            eng.dma_start(
                                out=t[:],
                                in_=hs2[:, g : g + 8, ca * TC : (cb + 1) * TC],
                            )
                        for j in range(8):
                            hs_t[(ca, g + j)] = t[:, j * 1024 : j * 1024 + TC]
                            hs_t[(cb, g + j)] = t[:, j * 1024 + TC : (j + 1) * 1024]
                    return hs_t

                def rope(acc, dst, c, nm):
                    """dst = acc*cos + shift128(acc*sin2)."""
                    u = workp.tile([P, TC], bf16, tag="ru", name=f"ru_{nm}")
                    w = workp.tile([P, TC], bf16, tag="rw", name=f"rw_{nm}")
                    sslc = sin_sb[:, c * TC : (c + 1) * TC]
                    nc.vector.tensor_mul(u[64:128, :], acc[0:64, :], sslc[0:64, :])
                    nc.vector.tensor_mul(u[0:64, :], acc[64:128, :], sslc[64:128, :])
                    nc.vector.tensor_mul(w[:], acc[:], cos_sb[:, c * TC : (c + 1) * TC])
                    nc.vector.tensor_add(dst[:], w[:], u[:])

                def proj_chunk(c, hs_t, weave=None, kv_first=False):
                    # Q pass first in steady state (its RoPE then hides
                    # under the KV pass); KV-first for chunk 0 since the
                    # smaller wk load lets the PE start sooner.
                    def q_section():
                        aq01 = p2("p01", f"aq01_{c}")
                        aq23 = p2("p23", f"aq23_{c}")
                        qacc = [aq01[:, 0:TC], aq01[:, TC : 2 * TC],
                                aq23[:, 0:TC], aq23[:, TC : 2 * TC]]
                        for ht in range(HT):
                            for o in range(4):
                                nc.tensor.matmul(
                                    qacc[o],
                                    wq_sb[:, ht * 512 + o * P : ht * 512 + (o + 1) * P],
                                    hs_t[(c, ht)],
                                    start=(ht == 0), stop=(ht == HT - 1),
                                )
                            if ht == 3 and weave is not None:
                                weave()
                        for o in range(4):
                            rope(qacc[o],
                                 qT_sb[:, o * S + c * TC : o * S + (c + 1) * TC],
                                 c, f"q_{c}_{o}")

                    def kv_section():
                        kacc = p1("pa", f"kacc_{c}")
                        vacc = p1("pb", f"vacc_{c}")
                        for ht in range(HT):
                            nc.tensor.matmul(
                                kacc[:], wk_sb[:, ht * P : (ht + 1) * P], hs_t[(c, ht)],
                                start=(ht == 0), stop=(ht == HT - 1),
                            )
                            nc.tensor.matmul(
                                vacc[:], wv_sb[:, ht * P : (ht + 1) * P], hs_t[(c, ht)],
                                start=(ht == 0), stop=(ht == HT - 1),
                            )
                        rope(kacc[:], kT_sb[:, c * TC : (c + 1) * TC], c, f"k_{c}")
                        vtmp = workp.tile([P, TC], bf16, tag="vtmp", name=f"vtmp_{c}")
                        nc.scalar.copy(vtmp[:], vacc[:])
                        for j in range(4):
                            tp = pm.tile(
                                [P, P], bf16, tag=("pc", "pd")[j % 2], bufs=1,
                                padded_shape=[P, TC], name=f"vt_{c}_{j}",
                            )
                            nc.tensor.transpose(tp[:], vtmp[:, j * P : (j + 1) * P], id_sb[:])
                            nc.vector.tensor_copy(
                                vnat_sb[:, (c * 4 + j) * P : (c * 4 + j + 1) * P], tp[:]
                            )

                    if kv_first:
                        kv_section()
                        q_section()
                    else:
                        q_section()
                        kv_section()

                hs0 = emit_loads(0, 0, 1)
                # constants load behind the first-chunk critical loads
                nc.sync.dma_start(out=cos_sb[:], in_=cos_dram[:])
                nc.scalar.dma_start(out=sin_sb[:], in_=sin_dram[:])
                nc.sync.dma_start(out=mask_sb[:], in_=mask_dram[:])
                nc.sync.dma_start(out=mask2_sb[:, 0:TC], in_=mask_dram[:])
                nc.scalar.dma_start(out=mask2_sb[:, TC : 2 * TC], in_=mask_dram[:])
                nc.scalar.dma_start(out=id_sb[:], in_=id_dram[:])
                proj_chunk(0, hs0)
                hs1 = emit_loads(1, 2, 3)  # prefetch during attn0/attn1
                t0 = attn(0)
                proj_chunk(1, hs0, weave=t0)
                t1 = attn(1)
                proj_chunk(2, hs1, weave=t1)
                t2 = attn(2)
                proj_chunk(3, hs1, weave=t2)
                t3 = attn(3)
                t3()

            # ---- Output projection: per-chunk passes in AllGather order.
            # One [128, 2048] read (4KB rows) per rank covers 4 heads; Wo
            # resident via two big contiguous DMAs.
            with (
                tc.tile_pool(name="wo", bufs=1) as wop,
                tc.tile_pool(name="workC", bufs=2) as workc,
            ):
                wo_sb = wop.tile([P, HT * NQH * HD], bf16)
                half = HT * NQH * HD // 2
                nc.sync.dma_start(out=wo_sb[:, 0:half], in_=wo2[:, 0:half])
                nc.scalar.dma_start(
                    out=wo_sb[:, half : 2 * half], in_=wo2[:, half : 2 * half]
                )

                for c in range(NT):
                    y01 = p2("p01", f"y01_{c}")
                    y23 = p2("p23", f"y23_{c}")
                    ys = [y01[:, 0:TC], y01[:, TC : 2 * TC],
                          y23[:, 0:TC], y23[:, TC : 2 * TC]]
                    for r in range(N_CORES):
                        sup = workc.tile([P, NQH * TC], bf16, tag="ag",
                                         bufs=4, name=f"ag_{c}_{r}")
                        eng = nc.sync if r % 2 == 0 else nc.scalar
                        eng.dma_start(
                            out=sup[:], in_=ag_out[c][r * P : (r + 1) * P, :]
                        )
                        for h in range(NQH):
                            ot = r * NQH + h
                            for o in range(4):
                                nc.tensor.matmul(
                                    ys[o],
                                    wo_sb[:, ot * 512 + o * P : ot * 512 + (o + 1) * P],
                                    sup[:, h * TC : (h + 1) * TC],
                                    start=(ot == 0), stop=(ot == HT - 1),
                                )
                    for o in range(4):
                        yo = workc.tile([P, TC], f32, tag="yo", bufs=4,
                                        name=f"yo_{c}_{o}")
                        if (c + o) % 2 == 0:
                            nc.scalar.copy(yo[:], ys[o])
                        else:
                            nc.vector.tensor_copy(yo[:], ys[o])
                        eng = nc.sync if (c + o) % 2 == 0 else nc.scalar
                        eng.dma_start(
                            out=out_ext[o * P : (o + 1) * P, c * TC : (c + 1) * TC],
                            in_=yo[:],
                        )

    nc.finalize()
    return nc


def _get_built():
    global _BUILT
    if _BUILT is None:
        _BUILT = _build()
    return _BUILT


def _pack_pm(mT):
    """[H, W] -> [128, (H/128)*W]: row p holds the concatenation over h-tiles
    of mT[ht*128+p, :], so every SBUF-destined DMA row is wide+contiguous."""
    h, w = mT.shape
    return np.ascontiguousarray(
        mT.reshape(h // P, P, w).transpose(1, 0, 2).reshape(P, (h // P) * w)
    )


def make_in_maps(hidden_states, Wq, Wk, Wv, Wo):
    bf = ml_dtypes.bfloat16
    hs = np.asarray(hidden_states, dtype=np.float32).reshape(S, H)
    hs2 = _pack_pm(np.ascontiguousarray(hs.T).astype(bf))
    in_maps = []
    for c in range(N_CORES):
        in_maps.append(
            {
                "hs2": hs2,
                "wq2": _pack_pm(np.ascontiguousarray(np.asarray(Wq)[c * 512 : (c + 1) * 512].T).astype(bf)),
                "wk2": _pack_pm(np.ascontiguousarray(np.asarray(Wk)[c * 128 : (c + 1) * 128].T).astype(bf)),
                "wv2": _pack_pm(np.ascontiguousarray(np.asarray(Wv)[c * 128 : (c + 1) * 128].T).astype(bf)),
                "wo2": _pack_pm(np.ascontiguousarray(np.asarray(Wo)[c * 512 : (c + 1) * 512].T).astype(bf)),
            }
        )
    return in_maps


def kernel(hidden_states, Wq, Wk, Wv, Wo):
    from concourse.bass_utils import run_bass_kernel_spmd

    nc = _get_built()
    in_maps = make_in_maps(hidden_states, Wq, Wk, Wv, Wo)
    r = run_bass_kernel_spmd(nc, in_maps, list(range(N_CORES)))
    yT = np.concatenate([r.results[c]["out"] for c in range(N_CORES)], axis=0)
    return np.ascontiguousarray(yT.T).reshape(1, S, H).astype(np.float32)


# revision 33
# speedup vs baseline: 1.0275x; 1.0275x over previous
"""Mistral attention (B=1, S=2048, H=4096, 32 q-heads / 8 kv-heads GQA,
RoPE, causal) on 8 trn2 NeuronCores.

Sharding: tensor-parallel by kv head. Core c owns kv head c, q heads
4c..4c+3, and Wo rows 512c..512c+512. Attention outputs are AllGathered
per 512-token chunk; each core then computes its 512-row slice of the
output projection.

Schedule: chunk-interleaved. Token chunks are processed in pairs
(0,1) then (2,3); for each pair the QKV projections run as two passes
(KV then Q) with each weight tile kept stationary on the PE for both
chunks' matmuls (halves LDWEIGHTS traffic). Attention for chunk c runs
right after its projections, and its AllGather fires immediately — the
four AllGathers cascade behind the remaining projection/attention
compute instead of bunching at the end. The output projection runs last
as two 2-chunk passes (weight tile stationary across both chunks).

Attention inner loop is kt-outer/head-inner so each K/V tile is loaded
once per chunk for 4 q-heads. Scores/exp/AV on masked diagonal tiles
are trimmed to live query columns (N = 512-128m). Softmax denominators
accumulate on the vector engine (elementwise over kt tiles) with a
single K=128 matmul per head at the end; the reciprocal is broadcast
across partitions with a K=1 f32r matmul. Softmax skips
max-subtraction (unit-scale inputs). Value path runs bf16; PSUM
accumulation fp32.
"""

import math

import ml_dtypes
import numpy as np

P = 128
S = 2048
H = 4096
HD = 128
NQH = 4  # q heads per core
TC = 512  # token chunk
NT = S // TC  # 4 chunks
HT = H // P  # 32 h tiles
N_CORES = 8
ROPE_THETA = 10000.0

_BUILT = None


def _rope_tables():
    """cosT/sin2T in [hd partition, token free] layout.

    sin2T is the sin table pre-shifted/signed so that
    q_rot = q*cosT + shift128(q*sin2T), where shift128 swaps the two
    64-partition halves.
    """
    inv_freq = 1.0 / (ROPE_THETA ** (np.arange(0, HD, 2, dtype=np.float64) / HD))
    t = np.arange(S, dtype=np.float64)
    freqs = np.outer(t, inv_freq)  # [S, 64]
    emb = np.concatenate([freqs, freqs], axis=1)  # [S, HD]
    cosT = np.cos(emb).T.astype(np.float32)  # [HD, S]
    sinT = np.sin(emb).T.astype(np.float32)
    sin2T = sinT.copy()
    sin2T[64:] = -sin2T[64:]
    return (
        np.ascontiguousarray(cosT).astype(ml_dtypes.bfloat16),
        np.ascontiguousarray(sin2T).astype(ml_dtypes.bfloat16),
    )


def _mask():
    """[128, 512] bf16: mask[i, j] = (j >= i). Diagonal tile m of a chunk
    uses mask[:, 0:512-128m] against query columns [128m, 512)."""
    i = np.arange(P)[:, None]
    j = np.arange(TC)[None, :]
    return np.ascontiguousarray((j >= i).astype(np.float32)).astype(ml_dtypes.bfloat16)


def _build():
    import concourse.bacc as bacc
    import concourse.mybir as mybir
    import concourse.tile as tile

    f32 = mybir.dt.float32
    f32r = mybir.dt.float32r
    bf16 = mybir.dt.bfloat16

    nc = bacc.Bacc(
        "TRN2", target_bir_lowering=False, debug=False, num_devices=N_CORES
    )

    # Host-side repacked layouts: partition-major [128, ...] with wide
    # contiguous rows so DMA descriptors are 2KB+ (1KB rows cap a DMA
    # queue at ~90GB/s; the kernel front is load-bound otherwise).
    hs2 = nc.declare_dram_parameter("hs2", [P, HT, S], bf16, isOutput=False)
    wq2 = nc.declare_dram_parameter("wq2", [P, HT * NQH * HD], bf16, isOutput=False)
    wk2 = nc.declare_dram_parameter("wk2", [P, HT * HD], bf16, isOutput=False)
    wv2 = nc.declare_dram_parameter("wv2", [P, HT * HD], bf16, isOutput=False)
    wo2 = nc.declare_dram_parameter("wo2", [P, HT * NQH * HD], bf16, isOutput=False)
    out_ext = nc.declare_dram_parameter("out", [NQH * HD, S], bf16, isOutput=True)

    cosT_np, sin2T_np = _rope_tables()
    cos_dram = nc.inline_tensor(cosT_np, name="cosT")
    sin_dram = nc.inline_tensor(sin2T_np, name="sin2T")
    mask_dram = nc.inline_tensor(_mask(), name="mask")
    id_dram = nc.inline_tensor(np.eye(P).astype(ml_dtypes.bfloat16), name="ident")

    # rank-major AllGather payload: [hd, (head, token)] per rank, so the
    # o-proj phase can read one rank's 4 heads as a single 4KB-row DMA
    ag_in = [nc.dram_tensor(f"ag_in{c}", [P, NQH * TC], bf16) for c in range(NT)]
    ag_out = [
        nc.dram_tensor(f"ag_out{c}", [N_CORES * P, NQH * TC], bf16, addr_space="Shared")
        for c in range(NT)
    ]

    Exp = mybir.ActivationFunctionType.Exp
    SCALE = 1.0 / math.sqrt(HD)

    with tile.TileContext(nc) as tc:
        with (
            tc.tile_pool(name="const", bufs=1) as constp,
            tc.tile_pool(name="qkvout", bufs=1) as qp,
            tc.tile_pool(name="pmain", bufs=1, space="PSUM") as pm,
        ):
            # constants
            cos_sb = constp.tile([P, S], bf16)
            sin_sb = constp.tile([P, S], bf16)
            mask_sb = constp.tile([P, TC], bf16)
            mask2_sb = constp.tile([P, 2 * TC], bf16)
            ones_sb = constp.tile([P, 1], bf16)
            onesrow_sb = constp.tile([1, P], bf16)
            id_sb = constp.tile([P, P], bf16)
            nc.gpsimd.memset(ones_sb[:], 1.0)
            nc.gpsimd.memset(onesrow_sb[:], 1.0)

            # persistent qkv outputs (bf16: PE runs bf16 at full rate)
            qT_sb = qp.tile([P, NQH * S], bf16)  # [hd, (head, t)]
            kT_sb = qp.tile([P, S], bf16)
            vnat_sb = qp.tile([P, S], bf16)  # [t%128, (ttile, hd)]

            # PSUM: 8 banks as two 2-bank tiles (p01, p23) and four 1-bank
            # tiles (pa..pd). Explicit tags keep cross-phase deps per-bank.
            def p2(tag, name):
                return pm.tile([P, 2 * TC], f32, tag=tag, bufs=1, name=name)

            def p1(tag, name):
                return pm.tile([P, TC], f32, tag=tag, bufs=1, name=name)

            with (
                tc.tile_pool(name="wqkv", bufs=1) as wp,
                tc.tile_pool(name="hsp", bufs=4) as hsp,
                tc.tile_pool(name="workA", bufs=2) as workp,
            ):
                wq_sb = wp.tile([P, HT * NQH * HD], bf16)
                wk_sb = wp.tile([P, HT * HD], bf16)
                wv_sb = wp.tile([P, HT * HD], bf16)

                def attn(c):
                    """Attention for chunk c + its AllGather."""
                    nkt = 4 * (c + 1)
                    avt = ["pa", "pb", "pc", "pd"]
                    av = [p1(avt[h], f"av_{c}_{h}") for h in range(NQH)]
                    ds = [
                        workp.tile([P, TC], bf16, tag=f"ds{h}", bufs=1,
                                   name=f"ds_{c}_{h}")
                        for h in range(NQH)
                    ]
                    pend = None  # (exs, coff, ncols, kt) awaiting AV matmuls

                    def emit_av(p):
                        exv, coff, kt = p
                        for h in range(NQH):
                            nc.tensor.matmul(
                                av[h][:, coff:TC],
                                vnat_sb[:, kt * P : (kt + 1) * P],
                                exv[h],
                                start=(kt == 0),
                                stop=(kt == nkt - 1),
                            )

                    for kt in range(nkt):
                        m = kt - 4 * c
                        ncols = TC - 128 * m if m > 0 else TC
                        coff = TC - ncols
                        scp = p2("p01", f"scp_{c}_{kt}")
                        scq = p2("p23", f"scq_{c}_{kt}")
                        halves = [
                            scp[:, 0:TC], scp[:, TC : 2 * TC],
                            scq[:, 0:TC], scq[:, TC : 2 * TC],
                        ]
                        for h in range(NQH):
                            nc.tensor.matmul(
                                halves[h][:, coff:TC],
                                kT_sb[:, kt * P : (kt + 1) * P],
                                qT_sb[:, h * S + c * TC + coff : h * S + (c + 1) * TC],
                                start=True,
                                stop=True,
                            )
                        if pend is not None:
                            emit_av(pend)
                        ex01 = workp.tile([P, 2 * TC], bf16, tag="ex", bufs=4,
                                          name=f"ex01_{c}_{kt}")
                        ex23 = workp.tile([P, 2 * TC], bf16, tag="ex", bufs=4,
                                          name=f"ex23_{c}_{kt}")
                        if coff == 0:
                            nc.scalar.activation(ex01[:], scp[:], Exp, scale=SCALE)
                            nc.scalar.activation(ex23[:], scq[:], Exp, scale=SCALE)
                        else:
                            for ex, sc in ((ex01, scp), (ex23, scq)):
                                nc.scalar.activation(
                                    ex[:, 0:ncols], sc[:, coff:TC], Exp, scale=SCALE
                                )
                                nc.scalar.activation(
                                    ex[:, TC : TC + ncols], sc[:, TC + coff : 2 * TC],
                                    Exp, scale=SCALE,
                                )
                        exv = [ex01[:, 0:ncols], ex01[:, TC : TC + ncols],
                               ex23[:, 0:ncols], ex23[:, TC : TC + ncols]]
                        if m == 0:
                            nc.vector.tensor_mul(ex01[:], ex01[:], mask2_sb[:])
                            nc.vector.tensor_mul(ex23[:], ex23[:], mask2_sb[:])
                        elif m > 0:
                            for h in range(NQH):
                                nc.vector.tensor_mul(
                                    exv[h], exv[h], mask_sb[:, 0:ncols]
                                )
                        for h in range(NQH):
                            if kt == 0:
                                nc.vector.tensor_copy(ds[h][:], exv[h])
                            else:
                                nc.vector.tensor_add(
                                    ds[h][:, coff:TC], ds[h][:, coff:TC], exv[h]
                                )
                        pend = (exv, coff, kt)
                    emit_av(pend)

                    # per-head tail: dn -> 1/dn -> broadcast -> normalize.
                    # dn/bc live in p01/p23 halves (the next phase needs
                    # those banks only at its Q pass, ~27us later). Heads
                    # 0-1 emit here; heads 2-3 + the AllGather return as a
                    # closure the caller weaves into the next phase's
                    # instruction stream (avoids PE-FIFO head-of-line).
                    avss = []
                    for h in range(NQH):
                        avs = workp.tile([P, TC], f32, tag=f"avs{h}", bufs=1,
                                         name=f"avs_{c}_{h}")
                        nc.scalar.copy(avs[:], av[h][:])
                        avss.append(avs)

                    def tail_pair(pair):
                        dnt = {}
                        for h in pair:
                            t = p1(("pc", "pd")[h % 2], f"dn_{c}_{h}")
                            nc.tensor.matmul(
                                t[0:1, :], ones_sb[:], ds[h][:],
                                start=True, stop=True,
                            )
                            dnt[h] = t
                        rcbs = {}
                        for h in pair:
                            rc = workp.tile([1, TC], f32, tag="rc", bufs=4,
                                            name=f"rc_{c}_{h}")
                            nc.vector.reciprocal_approx_fast(rc[:], dnt[h][0:1, :])
                            rcb = workp.tile([1, TC], bf16, tag="rcb", bufs=4,
                                             name=f"rcb_{c}_{h}")
                            nc.vector.tensor_copy(rcb[:], rc[:])
                            rcbs[h] = rcb
                        bct = {}
                        for h in pair:
                            t = p1(("pc", "pd")[h % 2], f"bc_{c}_{h}")
                            nc.tensor.matmul(
                                t[:], onesrow_sb[:], rcbs[h][:],
                                start=True, stop=True,
                            )
                            bct[h] = t
                        for h in pair:
                            ao = workp.tile([P, TC], bf16, tag="ao", bufs=4,
                                            name=f"ao_{c}_{h}")
                            nc.vector.tensor_mul(
                                ao[:], avss[h][:], bct[h][:]
                            )
                            nc.sync.dma_start(
                                out=ag_in[c][:, h * TC : (h + 1) * TC], in_=ao[:]
                            )

                    tail_pair((0, 1))

                    def finish():
                        tail_pair((2, 3))
                        nc.gpsimd.collective_compute(
                            "AllGather",
                            mybir.AluOpType.bypass,
                            ins=[ag_in[c][:]],
                            outs=[ag_out[c][:]],
                            replica_groups=[list(range(N_CORES))],
                        )

                    return finish

                def emit_loads(pi, ca, cb):
                    """hs for the pair: four [128, 8x1024] supertile DMAs
                    (2KB rows, each dma_start split across 16 SDMA engines).
                    Weights (first pair only): single big contiguous DMAs."""
                    hs_t = {}
                    for g in range(0, HT, 8):
                        t = hsp.tile([P, 8 * 2 * TC], bf16, tag="hs",
                                     name=f"hs_{ca}_{g}")
                        eng = nc.scalar if pi else (nc.sync if g % 16 == 0 else nc.scalar)
                        if pi == 0 and g == 0:
                            # startup cascade: Q pass leads, so wq quarters
                            # lead sync while hs streams on scalar
                            qq = HT * NQH * HD // 4
                            nc.sync.dma_start(out=wq_sb[:, 0:qq], in_=wq2[:, 0:qq])
                            nc.scalar.dma_start(
                                out=t[:, 0 : 4 * 1024],
                                in_=hs2[:, 0:4, ca * TC : (cb + 1) * TC],
                            )
                            nc.sync.dma_start(
                                out=wq_sb[:, qq : 2 * qq], in_=wq2[:, qq : 2 * qq]
                            )
                            nc.scalar.dma_start(
                                out=t[:, 4 * 1024 : 8 * 1024],
                                in_=hs2[:, 4:8, ca * TC : (cb + 1) * TC],
                            )
                        elif pi == 0 and g == 8:
                            qq = HT * NQH * HD // 4
                            nc.sync.dma_start(out=t[:], in_=hs2[:, 8:16, ca * TC : (cb + 1) * TC])
                            nc.scalar.dma_start(
                                out=wq_sb[:, 2 * qq : 3 * qq],
                                in_=wq2[:, 2 * qq : 3 * qq],
                            )
                            nc.scalar.dma_start(out=wk_sb[:], in_=wk2[:])
                        elif pi == 0 and g == 16:
                            qq = HT * NQH * HD // 4
                            nc.scalar.dma_start(out=t[:], in_=hs2[:, 16:24, ca * TC : (cb + 1) * TC])
                            nc.sync.dma_start(
                                out=wq_sb[:, 3 * qq : 4 * qq],
                                in_=wq2[:, 3 * qq : 4 * qq],
                            )
                            nc.sync.dma_start(out=wv_sb[:], in_=wv2[:])
                        else:
                            eng.dma_start(
                                out=t[:],
                                in_=hs2[:, g : g + 8, ca * TC : (cb + 1) * TC],
                            )
                        for j in range(8):
                            hs_t[(ca, g + j)] = t[:, j * 1024 : j * 1024 + TC]
                            hs_t[(cb, g + j)] = t[:, j * 1024 + TC : (j + 1) * 1024]
                        if pi == 0 and g < 16:
                            half = HT * NQH * HD // 2
                            weng = nc.sync if g == 0 else nc.scalar
                            weng.dma_start(
                                out=wq_sb[:, g // 8 * half : (g // 8 + 1) * half],
                                in_=wq2[:, g // 8 * half : (g // 8 + 1) * half],
                            )
                    return hs_t

                def rope(acc, dst, c, nm):
                    """dst = acc*cos + shift128(acc*sin2)."""
                    u = workp.tile([P, TC], bf16, tag="ru", name=f"ru_{nm}")
                    w = workp.tile([P, TC], bf16, tag="rw", name=f"rw_{nm}")
                    sslc = sin_sb[:, c * TC : (c + 1) * TC]
                    nc.vector.tensor_mul(u[64:128, :], acc[0:64, :], sslc[0:64, :])
                    nc.vector.tensor_mul(u[0:64, :], acc[64:128, :], sslc[64:128, :])
                    nc.vector.tensor_mul(w[:], acc[:], cos_sb[:, c * TC : (c + 1) * TC])
                    nc.vector.tensor_add(dst[:], w[:], u[:])

                def proj_chunk(c, hs_t, weave=None, kv_first=False):
                    # Q pass first in steady state (its RoPE then hides
                    # under the KV pass); KV-first for chunk 0 since the
                    # smaller wk load lets the PE start sooner.
                    def q_section():
                        aq01 = p2("p01", f"aq01_{c}")
                        aq23 = p2("p23", f"aq23_{c}")
                        qacc = [aq01[:, 0:TC], aq01[:, TC : 2 * TC],
                                aq23[:, 0:TC], aq23[:, TC : 2 * TC]]
                        for ht in range(HT):
                            for o in range(4):
                                nc.tensor.matmul(
                                    qacc[o],
                                    wq_sb[:, ht * 512 + o * P : ht * 512 + (o + 1) * P],
                                    hs_t[(c, ht)],
                                    start=(ht == 0), stop=(ht == HT - 1),
                                )
                            if ht == 3 and weave is not None:
                                weave()
                        for o in range(4):
                            rope(qacc[o],
                                 qT_sb[:, o * S + c * TC : o * S + (c + 1) * TC],
                                 c, f"q_{c}_{o}")

                    def kv_section():
                        kacc = p1("pa", f"kacc_{c}")
                        vacc = p1("pb", f"vacc_{c}")
                        for ht in range(HT):
                            nc.tensor.matmul(
                                kacc[:], wk_sb[:, ht * P : (ht + 1) * P], hs_t[(c, ht)],
                                start=(ht == 0), stop=(ht == HT - 1),
                            )
                            nc.tensor.matmul(
                                vacc[:], wv_sb[:, ht * P : (ht + 1) * P], hs_t[(c, ht)],
                                start=(ht == 0), stop=(ht == HT - 1),
                            )
                        rope(kacc[:], kT_sb[:, c * TC : (c + 1) * TC], c, f"k_{c}")
                        vtmp = workp.tile([P, TC], bf16, tag="vtmp", name=f"vtmp_{c}")
                        nc.scalar.copy(vtmp[:], vacc[:])
                        for j in range(4):
                            tp = pm.tile(
                                [P, P], bf16, tag=("pc", "pd")[j % 2], bufs=1,
                                padded_shape=[P, TC], name=f"vt_{c}_{j}",
                            )
                            nc.tensor.transpose(tp[:], vtmp[:, j * P : (j + 1) * P], id_sb[:])
                            nc.vector.tensor_copy(
                                vnat_sb[:, (c * 4 + j) * P : (c * 4 + j + 1) * P], tp[:]
                            )

                    if kv_first:
                        kv_section()
                        q_section()
                    else:
                        q_section()
                        kv_section()

                hs0 = emit_loads(0, 0, 1)
                # constants load behind the first-chunk critical loads
                nc.sync.dma_start(out=cos_sb[:], in_=cos_dram[:])
                nc.scalar.dma_start(out=sin_sb[:], in_=sin_dram[:])
                nc.sync.dma_start(out=mask_sb[:], in_=mask_dram[:])
                nc.sync.dma_start(out=mask2_sb[:, 0:TC], in_=mask_dram[:])
                nc.scalar.dma_start(out=mask2_sb[:, TC : 2 * TC], in_=mask_dram[:])
                nc.scalar.dma_start(out=id_sb[:], in_=id_dram[:])
                proj_chunk(0, hs0)
                hs1 = emit_loads(1, 2, 3)  # prefetch during attn0/attn1
                t0 = attn(0)
                proj_chunk(1, hs0, weave=t0)
                t1 = attn(1)
                proj_chunk(2, hs1, weave=t1)
                t2 = attn(2)
                proj_chunk(3, hs1, weave=t2)
                t3 = attn(3)
                t3()

            # ---- Output projection: per-chunk passes in AllGather order.
            # One [128, 2048] read (4KB rows) per rank covers 4 heads; Wo
            # resident via two big contiguous DMAs.
            with (
                tc.tile_pool(name="wo", bufs=1) as wop,
                tc.tile_pool(name="workC", bufs=2) as workc,
            ):
                wo_sb = wop.tile([P, HT * NQH * HD], bf16)
                half = HT * NQH * HD // 2
                nc.sync.dma_start(out=wo_sb[:, 0:half], in_=wo2[:, 0:half])
                nc.scalar.dma_start(
                    out=wo_sb[:, half : 2 * half], in_=wo2[:, half : 2 * half]
                )

                for c in range(NT):
                    y01 = p2("p01", f"y01_{c}")
                    y23 = p2("p23", f"y23_{c}")
                    ys = [y01[:, 0:TC], y01[:, TC : 2 * TC],
                          y23[:, 0:TC], y23[:, TC : 2 * TC]]
                    for r in range(N_CORES):
                        sup = workc.tile([P, NQH * TC], bf16, tag="ag",
                                         bufs=4, name=f"ag_{c}_{r}")
                        eng = nc.sync if r % 2 == 0 else nc.scalar
                        eng.dma_start(
                            out=sup[:], in_=ag_out[c][r * P : (r + 1) * P, :]
                        )
                        for h in range(NQH):
                            ot = r * NQH + h
                            for o in range(4):
                                nc.tensor.matmul(
                                    ys[o],
                                    wo_sb[:, ot * 512 + o * P : ot * 512 + (o + 1) * P],
                                    sup[:, h * TC : (h + 1) * TC],
                                    start=(ot == 0), stop=(ot == HT - 1),
                                )
                    for o in range(4):
                        yo = workc.tile([P, TC], bf16, tag="yo", bufs=4,
                                        name=f"yo_{c}_{o}")
                        if (c + o) % 2 == 0:
                            nc.scalar.copy(yo[:], ys[o])
                        else:
                            nc.vector.tensor_copy(yo[:], ys[o])
                        eng = nc.sync if (c + o) % 2 == 0 else nc.scalar
                        eng.dma_start(
                            out=out_ext[o * P : (o + 1) * P, c * TC : (c + 1) * TC],
                            in_=yo[:],
                        )

    nc.finalize()
    return nc


def _get_built():
    global _BUILT
    if _BUILT is None:
        _BUILT = _build()
    return _BUILT


def _pack_pm(mT):
    """[H, W] -> [128, (H/128)*W]: row p holds the concatenation over h-tiles
    of mT[ht*128+p, :], so every SBUF-destined DMA row is wide+contiguous."""
    h, w = mT.shape
    return np.ascontiguousarray(
        mT.reshape(h // P, P, w).transpose(1, 0, 2).reshape(P, (h // P) * w)
    )


def make_in_maps(hidden_states, Wq, Wk, Wv, Wo):
    bf = ml_dtypes.bfloat16
    hs = np.asarray(hidden_states, dtype=np.float32).reshape(S, H)
    hs2 = _pack_pm(np.ascontiguousarray(hs.T).astype(bf))
    in_maps = []
    for c in range(N_CORES):
        in_maps.append(
            {
                "hs2": hs2,
                "wq2": _pack_pm(np.ascontiguousarray(np.asarray(Wq)[c * 512 : (c + 1) * 512].T).astype(bf)),
                "wk2": _pack_pm(np.ascontiguousarray(np.asarray(Wk)[c * 128 : (c + 1) * 128].T).astype(bf)),
                "wv2": _pack_pm(np.ascontiguousarray(np.asarray(Wv)[c * 128 : (c + 1) * 128].T).astype(bf)),
                "wo2": _pack_pm(np.ascontiguousarray(np.asarray(Wo)[c * 512 : (c + 1) * 512].T).astype(bf)),
            }
        )
    return in_maps


def kernel(hidden_states, Wq, Wk, Wv, Wo):
    from concourse.bass_utils import run_bass_kernel_spmd

    nc = _get_built()
    in_maps = make_in_maps(hidden_states, Wq, Wk, Wv, Wo)
    r = run_bass_kernel_spmd(nc, in_maps, list(range(N_CORES)))
    yT = np.concatenate([r.results[c]["out"] for c in range(N_CORES)], axis=0)
    return np.ascontiguousarray(yT.T).reshape(1, S, H).astype(np.float32)
